# revision 68
# baseline (speedup 1.0000x reference)
"""Trainium2 Bass kernel for nn_CameraFrequency.

Reference computation:
    freq[f]    = L(f) @ diag(exp(D(f))) @ U(f)              [32,4,4]
    m5[b,c,f]  = freq[f] @ matrix[b,c]                      [4,8,32,4,4]
    feats      : [B=4, N=16, S=4096, FD=128] viewed as [b,n,c,p,f,j]
                 with S = C(8) * P(512), FD = F(32) * 4
    out[b,n,c,p,f,i] = sum_j m5[b,c,f,i,j] * feats[b,n,c,p,f,j]

Strategy:
  * Host precomputes, per (b,c), the 128x128 block-diagonal matrix
        W2[b,c, 4f+j, 4f+i] = m5[b,c,f,i,j]
    so that for a position row x (128-wide), y = x @ W2[b,c].
  * Data-parallel over the 64 (b,n) pairs: 8 cores x 8 heads.  Each core
    owns a single b, so it only needs W2[b] ([8,128,128], 512 KB), which
    the host appends to the first input DMA group.
  * Per-core kernel: stream feats in natural layout [pos, fd] tiles of
    [128,128]; transpose on the PE (fd -> partitions); matmul with
    lhsT = xT tile (so y = x @ W2 comes out in natural [pos, fd] layout);
    ACT copies xT PSUM->SBUF, DVE copies y PSUM->SBUF; DMA out.
    Memory-bound: 16 MiB in + 16 MiB out per core at ~360 GB/s
    -> ~93 us floor per core.

Toolchain note: this walrus build accepts at most ONE sync wait per
instruction (any engine, including the final drain).  Tile's scheduler
freely attaches several.  `_split_waits` post-processes the serialized
BIR: every instruction keeps its last wait and the rest move onto
preceding single-wait NoOps on the same engine queue, which is
semantically identical (sequencers execute in order).
"""

import os
import numpy as np

B, N, S, FD = 4, 16, 4096, 128
NF, DSZ = 32, 4
C = 8            # chunks along S (matrix's second dim)
PCHUNK = S // C  # 512 positions per chunk
NCORES = 8
HPC = (B * N) // NCORES  # heads per core = 8
GRP = 2                  # heads per DMA group
NGRP = HPC // GRP        # 4 groups
TPH = S // 128           # 32 pos-tiles per head
TPC = PCHUNK // 128      # 4 pos-tiles per chunk
# W2 appended as [C, 2, 128, 128] with zeroed second halves, so the device
# can round it to float32r with one copy and feed [128, 256] moving operands
WROWS = 2 * C * FD       # 2048 rows of appended W2 data

# knobs (test.py may override before calling kernel())
PROFILE = False
TRACE_DIR = None
LAST_EXEC_NS = None
LAST_RESULTS = None

_CACHED = {}


def _build_w2(matrix, L_params, D_params, U_params):
    """Per-(b,c) 128x128 block-diagonal matrices, numpy fp32."""
    L_params = np.asarray(L_params, np.float32)
    D_params = np.asarray(D_params, np.float32)
    U_params = np.asarray(U_params, np.float32)
    matrix = np.asarray(matrix, np.float32)

    n = L_params.shape[0]
    eye = np.eye(DSZ, dtype=np.float32)
    L = np.tile(eye[None], (n, 1, 1))
    L[:, 1, 0] = L_params[:, 0]
    L[:, 2, 0] = L_params[:, 1]
    L[:, 2, 1] = L_params[:, 2]
    L[:, 3, 0] = L_params[:, 3]
    L[:, 3, 1] = L_params[:, 4]
    L[:, 3, 2] = L_params[:, 5]
    U = np.tile(eye[None], (n, 1, 1))
    U[:, 0, 1] = U_params[:, 0]
    U[:, 0, 2] = U_params[:, 1]
    U[:, 0, 3] = U_params[:, 2]
    U[:, 1, 2] = U_params[:, 3]
    U[:, 1, 3] = U_params[:, 4]
    U[:, 2, 3] = U_params[:, 5]
    freq = np.einsum('fij,fj,fjk->fik', L, np.exp(D_params), U).astype(np.float32)
    # m5[b,c,f,i,j] = sum_k freq[f,i,k] * matrix[b,c,k,j]
    m5 = np.einsum('fik,bckj->bcfij', freq, matrix).astype(np.float32)
    w2 = np.zeros((B, C, FD, FD), np.float32)
    for f in range(NF):
        # W2[b,c, 4f+j, 4f+i] = m5[b,c,f,i,j]
        w2[:, :, 4 * f:4 * f + 4, 4 * f:4 * f + 4] = np.swapaxes(m5[:, :, f], -1, -2)
    return w2


def _split_waits(bir: dict) -> dict:
    """Walrus (this build) allows one sync wait per instruction: keep the
    last wait on each instruction and hoist the rest onto preceding
    single-wait NoOps on the same engine queue."""
    for fn in bir["functions"]:
        for blk in fn["blocks"]:
            out = []
            for inst in blk["instructions"]:
                si = inst.get("sync_info")
                waits = (si or {}).get("on_wait") or []
                if len(waits) > 1:
                    for k, w in enumerate(waits[:-1]):
                        out.append({
                            "engine": inst["engine"],
                            "ins": [],
                            "outs": [],
                            "name": f"{inst['name']}-w{k}",
                            "opcode": "NoOp",
                            "sync_info": {"on_update": [], "on_wait": [w]},
                        })
                    si["on_wait"] = [waits[-1]]
                out.append(inst)
            blk["instructions"] = out
    return bir


def _build_module():
    import orjson
    import concourse.bass as bass
    import concourse.mybir as mybir
    from concourse import tile

    f32 = mybir.dt.float32
    f32r = mybir.dt.float32r
    nc = bass.Bass()

    # Inputs are float32r-typed (same bits as fp32): satisfies the BIR
    # verifier's "rounded to FP32r" producer rule and selects the PE's
    # fast single-pass matmul mode.
    x = nc.dram_tensor("x", [HPC, S, FD], f32r, kind="ExternalInput")
    # wc = [C pairs of (W2[c] | zeros)] + [identity | zeros]
    wc = nc.dram_tensor("wc", [(2 * C + 2) * FD, FD], f32r,
                        kind="ExternalInput")
    y = nc.dram_tensor("y", [HPC, S, FD], f32, kind="ExternalOutput")

    GT = GRP * TPH  # 64 pos-tiles per input group (2 heads)

    with tile.TileContext(nc) as tc:
        with tc.tile_pool(name="consts", bufs=1) as cpool, \
             tc.tile_pool(name="ioxa", bufs=2) as xpool_a, \
             tc.tile_pool(name="ioxb", bufs=2) as xpool_b, \
             tc.tile_pool(name="ioy", bufs=2) as ypool, \
             tc.tile_pool(name="xt", bufs=4) as xtpool, \
             tc.tile_pool(name="ps_xt", bufs=4, space="PSUM") as ps_xt, \
             tc.tile_pool(name="ps_y", bufs=4, space="PSUM") as ps_y:

            # small W+identity DMA first so compute can start early
            wc_sb = cpool.tile([128, 2 * C + 2, FD], f32r, tag="wc")
            nc.sync.dma_start(
                out=wc_sb, in_=wc.rearrange("(t p) f -> p t f", p=128))
            w_r = wc_sb[:, :2 * C, :]
            # [identity | zeros] as a [128, 256] moving operand: transposes
            # become REGULAR f32r matmuls (1 cyc/row at N>=256, and they
            # count as PE activity so the HAM clock-gate stays warm,
            # unlike transpose-mode)
            ident2 = wc_sb[:, 2 * C:, :].rearrange("p t f -> p (t f)")

            # 4 input groups (2 heads each) in UNIQUE buffers, issued up
            # front and split across the two HWDGE rings (SP + ACT) so the
            # transfers run in parallel and never wait on slot recycling.
            group_x = []
            for g in range(NGRP):
                pool = xpool_a if g % 2 == 0 else xpool_b
                eng = nc.sync if g % 2 == 0 else nc.scalar
                xt_ = pool.tile([128, GT, FD], f32r, tag=f"x{g % 2}")
                eng.dma_start(
                    out=xt_,
                    in_=x[GRP * g:GRP * (g + 1)].rearrange(
                        "n (t p) f -> p (n t) f", p=128))
                group_x.append(xt_)

            head_y = {}

            # Software-pipelined emission: interleave [transposes of chunk
            # k+1 | matmuls of chunk k] in the compile-time-fixed PE stream
            # so the PE never idles on the ACT PSUM->SBUF copy.
            def stage_a(k):
                h, c = divmod(k, C)
                if c == 0:
                    yt_ = ypool.tile([128, TPH, FD], f32, tag="y")
                    head_y[h] = yt_
                g, hh = divmod(h, GRP)
                tbase = hh * TPH + c * TPC
                x_sb = group_x[g]
                # two [128,512] PSUM tiles per chunk, each holding two
                # [xT_tile | zeros] transpose results side by side
                pair_ps = []
                for pr in range(TPC // 2):
                    xT_ps = ps_xt.tile([128, PCHUNK], f32, tag="xT")
                    for half in range(2):
                        u = 2 * pr + half
                        nc.tensor.matmul(
                            xT_ps[:, half * 256:half * 256 + 256],
                            lhsT=x_sb[:, tbase + u, :],
                            rhs=ident2,
                            start=True, stop=True)
                    pair_ps.append(xT_ps)
                # ACT copies round fp32 PSUM -> float32r SBUF, gathering
                # the useful halves
                xT_sb = xtpool.tile([128, PCHUNK], f32r, tag="xTs")
                for pr in range(TPC // 2):
                    nc.scalar.copy(
                        out=xT_sb[:, pr * 256:(pr + 1) * 256].rearrange(
                            "p (u hf) -> p u hf", u=2),
                        in_=pair_ps[pr].rearrange(
                            "p (u hf) -> p u hf", u=2)[:, :, :128])
                return (h, c, xT_sb)

            def stage_b(st):
                h, c, xT_sb = st
                y_sb = head_y[h]
                tbase = c * TPC
                wv = w_r[:, 2 * c:2 * c + 2, :].rearrange("p t f -> p (t f)")
                for pair in range(TPC // 2):
                    y_ps = ps_y.tile([128, PCHUNK], f32, tag="yps")
                    for half in range(2):
                        u = 2 * pair + half
                        nc.tensor.matmul(
                            y_ps[:, half * 256:half * 256 + 256],
                            lhsT=xT_sb[:, u * 128:(u + 1) * 128],
                            rhs=wv,
                            start=True, stop=True)
                    nc.vector.tensor_copy(
                        out=y_sb[:, tbase + 2 * pair:tbase + 2 * pair + 2, :],
                        in_=y_ps.rearrange(
                            "p (u hf) -> p u hf", u=2)[:, :, :128])
                if c == C - 1:  # last chunk of the head: stream it out
                    eng = nc.sync if h % 2 == 0 else nc.scalar
                    eng.dma_start(
                        out=y[h].rearrange("(t p) f -> p t f", p=128),
                        in_=y_sb)

            NCHUNK = HPC * C
            pending = stage_a(0)
            for k in range(1, NCHUNK):
                nxt = stage_a(k)
                stage_b(pending)
                pending = nxt
            stage_b(pending)

    orig_to_json_bytes = nc.to_json_bytes

    def patched_to_json_bytes():
        return orjson.dumps(_split_waits(orjson.loads(orig_to_json_bytes())))

    nc.to_json_bytes = patched_to_json_bytes
    return nc


def _get_module():
    if "nc" not in _CACHED:
        _CACHED["nc"] = _build_module()
    return _CACHED["nc"]


def kernel(feats, matrix, L_params, D_params, U_params):
    global LAST_EXEC_NS, LAST_RESULTS
    from concourse.bass_utils import run_bass_kernel_spmd

    feats = np.ascontiguousarray(np.asarray(feats, np.float32))
    w2 = _build_w2(matrix, L_params, D_params, U_params)

    nc = _get_module()

    in_maps = []
    for k in range(NCORES):
        b = k // (NCORES // B)            # 2 cores per b
        h0 = HPC * (k % (NCORES // B))    # head offset within b
        xf = feats[b, h0:h0 + HPC]
        # wc = [C pairs of (W2[c] | zeros)] + [identity | zeros]
        wrows = np.zeros((2 * C + 2, FD, FD), np.float32)
        wrows[0:2 * C:2] = w2[b]
        wrows[2 * C] = np.eye(FD, dtype=np.float32)
        in_maps.append({
            "x": np.ascontiguousarray(xf),
            "wc": np.ascontiguousarray(wrows.reshape((2 * C + 2) * FD, FD)),
        })

    kwargs = {}
    if PROFILE:
        kwargs["trace"] = True
        if TRACE_DIR:
            os.makedirs(TRACE_DIR, exist_ok=True)
            kwargs["tmpdir"] = TRACE_DIR

    res = run_bass_kernel_spmd(nc, in_maps, core_ids=list(range(NCORES)),
                               **kwargs)
    LAST_EXEC_NS = res.exec_time_ns
    LAST_RESULTS = res

    out = np.empty((B, N, S, FD), np.float32)
    for k in range(NCORES):
        b = k // (NCORES // B)
        h0 = HPC * (k % (NCORES // B))
        out[b, h0:h0 + HPC] = res.results[k]["y"]
    return out


# revision 70
# speedup vs baseline: 1.1623x; 1.1623x over previous
"""Trainium2 Bass kernel for nn_CameraFrequency.

Reference computation:
    freq[f]    = L(f) @ diag(exp(D(f))) @ U(f)              [32,4,4]
    m5[b,c,f]  = freq[f] @ matrix[b,c]                      [4,8,32,4,4]
    feats      : [B=4, N=16, S=4096, FD=128] viewed as [b,n,c,p,f,j]
                 with S = C(8) * P(512), FD = F(32) * 4
    out[b,n,c,p,f,i] = sum_j m5[b,c,f,i,j] * feats[b,n,c,p,f,j]

Strategy:
  * Host precomputes, per (b,c), the 128x128 block-diagonal matrix
        W2[b,c, 4f+j, 4f+i] = m5[b,c,f,i,j]
    so that for a position row x (128-wide), y = x @ W2[b,c].
  * Data-parallel over the 64 (b,n) pairs: 8 cores x 8 heads.  Each core
    owns a single b, so it only needs W2[b] ([8,128,128], 512 KB), which
    the host appends to the first input DMA group.
  * Per-core kernel: stream feats in natural layout [pos, fd] tiles of
    [128,128]; transpose on the PE (fd -> partitions); matmul with
    lhsT = xT tile (so y = x @ W2 comes out in natural [pos, fd] layout);
    ACT copies xT PSUM->SBUF, DVE copies y PSUM->SBUF; DMA out.
    Memory-bound: 16 MiB in + 16 MiB out per core at ~360 GB/s
    -> ~93 us floor per core.

Toolchain note: this walrus build accepts at most ONE sync wait per
instruction (any engine, including the final drain).  Tile's scheduler
freely attaches several.  `_split_waits` post-processes the serialized
BIR: every instruction keeps its last wait and the rest move onto
preceding single-wait NoOps on the same engine queue, which is
semantically identical (sequencers execute in order).
"""

import os
import numpy as np

B, N, S, FD = 4, 16, 4096, 128
NF, DSZ = 32, 4
C = 8            # chunks along S (matrix's second dim)
PCHUNK = S // C  # 512 positions per chunk
NCORES = 8
HPC = (B * N) // NCORES  # heads per core = 8
GRP = 2                  # heads per DMA group
NGRP = HPC // GRP        # 4 groups
TPH = S // 128           # 32 pos-tiles per head
TPC = PCHUNK // 128      # 4 pos-tiles per chunk
# W2 appended as [C, 2, 128, 128] with zeroed second halves, so the device
# can round it to float32r with one copy and feed [128, 256] moving operands
WROWS = 2 * C * FD       # 2048 rows of appended W2 data

# knobs (test.py may override before calling kernel())
PROFILE = False
TRACE_DIR = None
LAST_EXEC_NS = None
LAST_RESULTS = None

_CACHED = {}


def _build_w2(matrix, L_params, D_params, U_params):
    """Per-(b,c) 128x128 block-diagonal matrices, numpy fp32."""
    L_params = np.asarray(L_params, np.float32)
    D_params = np.asarray(D_params, np.float32)
    U_params = np.asarray(U_params, np.float32)
    matrix = np.asarray(matrix, np.float32)

    n = L_params.shape[0]
    eye = np.eye(DSZ, dtype=np.float32)
    L = np.tile(eye[None], (n, 1, 1))
    L[:, 1, 0] = L_params[:, 0]
    L[:, 2, 0] = L_params[:, 1]
    L[:, 2, 1] = L_params[:, 2]
    L[:, 3, 0] = L_params[:, 3]
    L[:, 3, 1] = L_params[:, 4]
    L[:, 3, 2] = L_params[:, 5]
    U = np.tile(eye[None], (n, 1, 1))
    U[:, 0, 1] = U_params[:, 0]
    U[:, 0, 2] = U_params[:, 1]
    U[:, 0, 3] = U_params[:, 2]
    U[:, 1, 2] = U_params[:, 3]
    U[:, 1, 3] = U_params[:, 4]
    U[:, 2, 3] = U_params[:, 5]
    freq = np.einsum('fij,fj,fjk->fik', L, np.exp(D_params), U).astype(np.float32)
    # m5[b,c,f,i,j] = sum_k freq[f,i,k] * matrix[b,c,k,j]
    m5 = np.einsum('fik,bckj->bcfij', freq, matrix).astype(np.float32)
    w2 = np.zeros((B, C, FD, FD), np.float32)
    for f in range(NF):
        # W2[b,c, 4f+j, 4f+i] = m5[b,c,f,i,j]
        w2[:, :, 4 * f:4 * f + 4, 4 * f:4 * f + 4] = np.swapaxes(m5[:, :, f], -1, -2)
    return w2


def _split_waits(bir: dict) -> dict:
    """Walrus (this build) allows one sync wait per instruction: keep the
    last wait on each instruction and hoist the rest onto preceding
    single-wait NoOps on the same engine queue."""
    for fn in bir["functions"]:
        for blk in fn["blocks"]:
            out = []
            for inst in blk["instructions"]:
                si = inst.get("sync_info")
                waits = (si or {}).get("on_wait") or []
                if len(waits) > 1:
                    for k, w in enumerate(waits[:-1]):
                        out.append({
                            "engine": inst["engine"],
                            "ins": [],
                            "outs": [],
                            "name": f"{inst['name']}-w{k}",
                            "opcode": "NoOp",
                            "sync_info": {"on_update": [], "on_wait": [w]},
                        })
                    si["on_wait"] = [waits[-1]]
                out.append(inst)
            blk["instructions"] = out
    return bir


def _build_module():
    import orjson
    import concourse.bass as bass
    import concourse.mybir as mybir
    from concourse import tile
    from concourse.masks import make_identity

    f32 = mybir.dt.float32
    f32r = mybir.dt.float32r
    nc = bass.Bass()

    # group 0 carries [2 heads of feats | W2 data (zero-padded pairs)]
    x0 = nc.dram_tensor("x0", [GRP * S + WROWS, FD], f32, kind="ExternalInput")
    xr = nc.dram_tensor("xr", [NGRP - 1, GRP * S, FD], f32,
                        kind="ExternalInput")
    y = nc.dram_tensor("y", [HPC, S, FD], f32, kind="ExternalOutput")

    GT = GRP * TPH          # 64 pos-tiles per group
    G0T = GT + 2 * C        # +16 W2 tiles in group 0 (zero-padded pairs)

    with tile.TileContext(nc) as tc:
        with tc.tile_pool(name="consts", bufs=1) as cpool, \
             tc.tile_pool(name="iox0", bufs=1) as x0pool, \
             tc.tile_pool(name="iox", bufs=2) as xpool, \
             tc.tile_pool(name="ioy", bufs=2) as ypool, \
             tc.tile_pool(name="xt", bufs=4) as xtpool, \
             tc.tile_pool(name="ps_xt", bufs=4, space="PSUM") as ps_xt, \
             tc.tile_pool(name="ps_y", bufs=4, space="PSUM") as ps_y:

            ident = cpool.tile([128, 128], f32, tag="ident")
            make_identity(nc, ident)

            x0_sb = x0pool.tile([128, G0T, FD], f32, tag="x0")
            nc.sync.dma_start(
                out=x0_sb, in_=x0.rearrange("(t p) f -> p t f", p=128))
            # W2 rounded to float32r; layout [128 j, (c, half), 128]: tile
            # 2c holds chunk c's block-diagonal matrix, tile 2c+1 zeros, so
            # [:, 2c:2c+2, :] is a [128, 256] moving operand for the f32r
            # 1 cyc/row matmul path.
            w_r = cpool.tile([128, 2 * C, FD], f32r, tag="w_r")
            nc.scalar.copy(out=w_r, in_=x0_sb[:, GT:, :])

            # Software-pipelined emission: the PE instruction stream is
            # fixed at compile time, so interleave [transposes of chunk
            # k+1 | matmuls of chunk k].  While chunk k's matmuls wait on
            # the ACT PSUM->SBUF copy, the PE runs chunk k+1's transposes
            # instead of idling.
            group_x = {}
            group_y = {}

            def stage_a(k):
                """transposes + rounding copy for chunk k; returns state"""
                g, rem = divmod(k, GRP * C)
                hh, c = divmod(rem, C)
                if rem == 0:
                    if g == 0:
                        group_x[g] = x0_sb
                    else:
                        xt_ = xpool.tile([128, GT, FD], f32, tag="x")
                        nc.sync.dma_start(
                            out=xt_,
                            in_=xr[g - 1].rearrange("(t p) f -> p t f",
                                                    p=128))
                        group_x[g] = xt_
                    yt_ = ypool.tile([128, GT, FD], f32, tag="y")
                    group_y[g] = yt_
                tbase = hh * TPH + c * TPC
                xT_ps = ps_xt.tile([128, PCHUNK], f32, tag="xT")
                for u in range(TPC):
                    nc.tensor.transpose(
                        xT_ps[:, u * 128:(u + 1) * 128],
                        group_x[g][:, tbase + u, :],
                        ident)
                # the PSUM->SBUF copy also rounds to float32r for the
                # matmul (the transposes themselves stay exact fp32)
                xT_sb = xtpool.tile([128, PCHUNK], f32r, tag="xTs")
                nc.scalar.copy(out=xT_sb, in_=xT_ps)
                return (g, c, tbase, xT_sb)

            def stage_b(st):
                """float32r matmuls + y copies (+ group out-DMA) of chunk"""
                g, c, tbase, xT_sb = st
                y_sb = group_y[g]
                # each matmul writes [y_tile | zeros-from-pad]; two share
                # one PSUM bank, DVE copies out the y halves.
                wv = w_r[:, 2 * c:2 * c + 2, :].rearrange("p t f -> p (t f)")
                for pair in range(TPC // 2):
                    y_ps = ps_y.tile([128, PCHUNK], f32, tag="yps")
                    for half in range(2):
                        u = 2 * pair + half
                        nc.tensor.matmul(
                            y_ps[:, half * 256:half * 256 + 256],
                            lhsT=xT_sb[:, u * 128:(u + 1) * 128],
                            rhs=wv,
                            start=True, stop=True)
                    nc.vector.tensor_copy(
                        out=y_sb[:, tbase + 2 * pair:tbase + 2 * pair + 2, :],
                        in_=y_ps.rearrange(
                            "p (u hf) -> p u hf", u=2)[:, :, :128])
                if tbase + TPC == GT:  # last chunk of the group
                    nc.scalar.dma_start(
                        out=y[GRP * g:GRP * (g + 1)].rearrange(
                            "n (t p) f -> p (n t) f", p=128),
                        in_=y_sb)

            NCHUNK = NGRP * GRP * C
            pending = stage_a(0)
            for k in range(1, NCHUNK):
                nxt = stage_a(k)
                stage_b(pending)
                pending = nxt
            stage_b(pending)

    orig_to_json_bytes = nc.to_json_bytes

    def patched_to_json_bytes():
        return orjson.dumps(_split_waits(orjson.loads(orig_to_json_bytes())))

    nc.to_json_bytes = patched_to_json_bytes
    return nc


def _get_module():
    if "nc" not in _CACHED:
        _CACHED["nc"] = _build_module()
    return _CACHED["nc"]


def kernel(feats, matrix, L_params, D_params, U_params):
    global LAST_EXEC_NS, LAST_RESULTS
    from concourse.bass_utils import run_bass_kernel_spmd

    feats = np.ascontiguousarray(np.asarray(feats, np.float32))
    w2 = _build_w2(matrix, L_params, D_params, U_params)

    nc = _get_module()

    in_maps = []
    for k in range(NCORES):
        b = k // (NCORES // B)            # 2 cores per b
        h0 = HPC * (k % (NCORES // B))    # head offset within b
        xf = feats[b, h0:h0 + HPC]
        # group 0 carries [2 heads | C pairs of (W2[c] | zeros)]
        wrows = np.zeros((C, 2, FD, FD), np.float32)
        wrows[:, 0] = w2[b]
        x0 = np.concatenate(
            [xf[0:GRP].reshape(GRP * S, FD),
             wrows.reshape(WROWS, FD)], axis=0)
        xr = xf[GRP:].reshape(NGRP - 1, GRP * S, FD)
        in_maps.append({
            "x0": np.ascontiguousarray(x0),
            "xr": np.ascontiguousarray(xr),
        })

    kwargs = {}
    if PROFILE:
        kwargs["trace"] = True
        if TRACE_DIR:
            os.makedirs(TRACE_DIR, exist_ok=True)
            kwargs["tmpdir"] = TRACE_DIR

    res = run_bass_kernel_spmd(nc, in_maps, core_ids=list(range(NCORES)),
                               **kwargs)
    LAST_EXEC_NS = res.exec_time_ns
    LAST_RESULTS = res

    out = np.empty((B, N, S, FD), np.float32)
    for k in range(NCORES):
        b = k // (NCORES // B)
        h0 = HPC * (k % (NCORES // B))
        out[b, h0:h0 + HPC] = res.results[k]["y"]
    return out
